# revision 1
# baseline (speedup 1.0000x reference)
"""NoisyLinear (factorized-noise nn.Module) Bass/Tile kernel for 8 TRN2 NeuronCores.

Math (per full-batch):
    out[b,o] = sum_i x[b,i]*wmu[o,i]                          (deterministic)
             + sum_i ws[o,i]*eps[b,o,i]*x[b,i]                (noisy)
             + bmu[o] + bs[o]*epsb[b,o]                       (biases)

Sharding: pure data-parallel over batch (B=256 -> 32 per core); weights and
biases replicated. eps (512 MiB total) dominates -> memory-bound.

Per-core kernel layout: o on partitions (4 o-tiles of 128), i on free dim.
  - det branch: PE matmul with transposed-loaded wmu^T and x^T.
  - noisy: per (b, o-tile): DVE pass1 t = eps*ws, DVE pass2 z = t * x_bcast
    with fused free-dim reduction (accum_out) -> noisy column [128,1].
    x_bcast ([128,1024] = x[b,:] replicated on all partitions) is built by a
    K=1 PE matmul: ones[1,128].T @ x_row[1,512].
  - biases: one fused tensor_scalar per o-tile on [128,32] epsb^T tiles.
  - final: add 3 terms [128,32], PE-transpose to [32,128], assemble [32,512].
"""

import numpy as np

import concourse.bass as bass
import concourse.tile as tile
from concourse import bacc, mybir
from concourse.bass import ts
from concourse.bass_utils import run_bass_kernel_spmd
from concourse.masks import make_identity

B, O, I = 256, 512, 1024
NCORES = 8
BS = B // NCORES  # 32 samples per core
OT = O // 128     # 4 o-tiles
KC = I // 128     # 8 i-chunks

FP = mybir.dt.float32
Alu = mybir.AluOpType


def _emit(nc, tc, loop_iters=0):
    # host pre-transposed aux layouts (tiny, replicated/per-shard) so every
    # device DMA moves wide contiguous bursts
    x = nc.dram_tensor("x", [BS, I], FP, kind="ExternalInput").ap()
    wmu_t = nc.dram_tensor("wmu_t", [I, O], FP, kind="ExternalInput").ap()
    bmu = nc.dram_tensor("bias_mu", [O], FP, kind="ExternalInput").ap()
    ws = nc.dram_tensor("weight_sigma", [O, I], FP, kind="ExternalInput").ap()
    bs = nc.dram_tensor("bias_sigma", [O], FP, kind="ExternalInput").ap()
    weps = nc.dram_tensor("weight_epsilon_batch", [BS, O, I], FP, kind="ExternalInput").ap()
    epsb_t = nc.dram_tensor("epsb_t", [O, BS], FP, kind="ExternalInput").ap()
    x_t = nc.dram_tensor("x_t", [I, BS], FP, kind="ExternalInput").ap()
    out = nc.dram_tensor("out", [BS, O], FP, kind="ExternalOutput").ap()

    import contextlib

    with (
        tc.tile_pool(name="const", bufs=1) as const_pool,
        tc.tile_pool(name="xrow", bufs=3) as xrow_pool,
        tc.tile_pool(name="eps", bufs=3) as eps_pool,
        tc.tile_pool(name="scr", bufs=3) as scr_pool,
        tc.tile_pool(name="acc", bufs=1) as acc_pool,
        tc.tile_pool(name="psum", bufs=1, space="PSUM") as psum_pool,
        tc.For_i(0, loop_iters, 1) if loop_iters else contextlib.nullcontext(),
    ):
        # ---- resident constants --------------------------------------------
        ws_all = const_pool.tile([128, OT, I], FP, name="ws_all")
        nc.sync.dma_start(ws_all[:], ws.rearrange("(ot p) i -> p ot i", p=128))

        # wmu^T chunks for PE: wmuT[p=i%128, kc, o] = wmu[o, kc*128+p]
        wmuT = const_pool.tile([128, KC, O], FP, name="wmuT")
        nc.sync.dma_start(wmuT[:], wmu_t.rearrange("(kc p) o -> p kc o", p=128))

        # x^T chunks: xT[p=i%128, kc, b] = x[b, kc*128+p]
        xT = const_pool.tile([128, KC, BS], FP, name="xT")
        nc.sync.dma_start(xT[:], x_t.rearrange("(kc p) b -> p kc b", p=128))

        bmu_col = const_pool.tile([128, OT], FP, name="bmu_col")
        nc.sync.dma_start(bmu_col[:], bmu.rearrange("(ot p) -> p ot", p=128))
        bs_col = const_pool.tile([128, OT], FP, name="bs_col")
        nc.sync.dma_start(bs_col[:], bs.rearrange("(ot p) -> p ot", p=128))

        # epsb^T: [o-part, ot, b]
        epsbT = const_pool.tile([128, OT, BS], FP, name="epsbT")
        nc.sync.dma_start(epsbT[:], epsb_t.rearrange("(ot p) b -> p ot b", p=128))

        ones_row = const_pool.tile([1, 128], FP, name="ones_row")
        nc.gpsimd.memset(ones_row[:], 1.0)

        ident = const_pool.tile([128, 128], FP, name="ident")
        make_identity(nc, ident[:])

        # ---- deterministic branch on PE: det[o,b] = sum_i wmu[o,i]x[b,i] ---
        det_sb = acc_pool.tile([128, OT, BS], FP, name="det_sb")
        for ot in range(OT):
            det_ps = psum_pool.tile([128, BS], FP, name="det_ps", tag="det_ps", bufs=2)
            for kc in range(KC):
                nc.tensor.matmul(
                    det_ps[:],
                    wmuT[:, kc, ts(ot, 128)],
                    xT[:, kc, :],
                    start=(kc == 0),
                    stop=(kc == KC - 1),
                )
            nc.scalar.copy(det_sb[:, ot, :], det_ps[:])

        # ---- bias term: bias_t[o,b] = epsb[b,o]*bs[o] + bmu[o] -------------
        bias_t = acc_pool.tile([128, OT, BS], FP, name="bias_t")
        for ot in range(OT):
            nc.vector.tensor_scalar(
                bias_t[:, ot, :],
                epsbT[:, ot, :],
                bs_col[:, ot : ot + 1],
                bmu_col[:, ot : ot + 1],
                Alu.mult,
                Alu.add,
            )

        # ---- noisy branch: 2 elementwise passes per b ----------------------
        # x_bcast built by PE (K=1 ones matmul) then copied PSUM->SBUF by the
        # idle ACT engine so both passes are SBUF-only.
        # pass1 (t = eps*x_bcast) is ONE [128, 4*I] op per sample (x_bcast
        # rides a stride-0 broadcast dim), split across DVE (1/3 of samples)
        # and Pool (2/3) so both engines finish together (~5.7 us/sample,
        # under the ~6.5 us/sample DMA floor). pass2 (z = t*ws + fused
        # free-dim reduce via accum_out) runs on DVE (~1.0 us/tile).
        noisy = acc_pool.tile([128, OT, BS], FP, name="noisy")
        tile_idx = 0
        for b in range(BS):
            xrow = xrow_pool.tile([1, I], FP, name="xrow", tag="xrow")
            nc.sync.dma_start(xrow[:], x[b : b + 1, :])

            # x_bcast[128, I] = x[b,:] on every partition (K=1 PE matmul)
            xb_ps = psum_pool.tile([128, I], FP, name="xb_ps", tag="xb_ps", bufs=2)
            for j in range(I // 512):
                nc.tensor.matmul(
                    xb_ps[:, ts(j, 512)],
                    ones_row[:],
                    xrow[:, ts(j, 512)],
                    start=True,
                    stop=True,
                )
            xb_sb = scr_pool.tile([128, I], FP, name="xb_sb", tag="xb_sb", bufs=3)
            nc.scalar.copy(xb_sb[:], xb_ps[:])

            # one batched DMA for all 4 o-tiles of sample b (2 MiB)
            eps_t = eps_pool.tile([128, OT, I], FP, name="eps_t", tag="eps_t")
            nc.sync.dma_start(eps_t[:], weps[b].rearrange("(ot p) i -> p ot i", p=128))

            import os

            variant = os.environ.get("KERNEL_VARIANT", "")
            for ot in range(OT):
                t = scr_pool.tile([128, I], FP, name="t", tag="t", bufs=6)
                if variant == "nopass1":
                    t = eps_t[:, ot, :]
                elif variant == "dve_all" or tile_idx % 18 < 7:
                    nc.vector.tensor_mul(t[:], eps_t[:, ot, :], xb_sb[:])
                    t = t[:]
                else:
                    nc.gpsimd.tensor_mul(t[:], eps_t[:, ot, :], xb_sb[:])
                    t = t[:]
                tile_idx += 1
                z = scr_pool.tile([128, I], FP, name="z", tag="z", bufs=6)
                nc.vector.scalar_tensor_tensor(
                    out=z[:],
                    in0=t,
                    scalar=1.0,
                    in1=ws_all[:, ot, :],
                    op0=Alu.bypass,
                    op1=Alu.mult,
                    accum_out=noisy[:, ot, b : b + 1],
                )

        # ---- combine + transpose back to [b, o] ----------------------------
        out_sb = acc_pool.tile([BS, O], FP, name="out_sb")
        for ot in range(OT):
            comb = scr_pool.tile([128, BS], FP, name="comb", tag="comb")
            nc.vector.tensor_add(comb[:], noisy[:, ot, :], det_sb[:, ot, :])
            comb2 = scr_pool.tile([128, BS], FP, name="comb2", tag="comb2")
            nc.vector.tensor_add(comb2[:], comb[:], bias_t[:, ot, :])
            tr_ps = psum_pool.tile([BS, 128], FP, name="tr_ps", tag="tr_ps", bufs=2)
            nc.tensor.transpose(tr_ps[:], comb2[:], ident[:])
            nc.scalar.copy(out_sb[:, ts(ot, 128)], tr_ps[:])

        nc.sync.dma_start(out[:], out_sb[:])


_CACHE = {}


def _build(loop_iters=0):
    key = ("nc", loop_iters)
    if key not in _CACHE:
        nc = bacc.Bacc(
            "TRN2",
            target_bir_lowering=False,
            debug=False,
            num_devices=NCORES,
        )
        with tile.TileContext(nc) as tc:
            _emit(nc, tc, loop_iters=loop_iters)
        nc.compile()
        _CACHE[key] = nc
    return _CACHE[key]


def _shard_inputs(inputs):
    arrs = {k: np.ascontiguousarray(np.asarray(v), dtype=np.float32) for k, v in inputs.items()}
    wmu_t = np.ascontiguousarray(arrs["weight_mu"].T)
    in_maps = []
    for c in range(NCORES):
        sl = slice(c * BS, (c + 1) * BS)
        x_sh = arrs["x"][sl]
        in_maps.append(
            {
                "x": x_sh,
                "x_t": np.ascontiguousarray(x_sh.T),
                "wmu_t": wmu_t,
                "bias_mu": arrs["bias_mu"],
                "weight_sigma": arrs["weight_sigma"],
                "bias_sigma": arrs["bias_sigma"],
                "weight_epsilon_batch": arrs["weight_epsilon_batch"][sl],
                "epsb_t": np.ascontiguousarray(arrs["bias_epsilon_batch"][sl].T),
            }
        )
    return in_maps


def kernel(**inputs) -> np.ndarray:
    nc = _build()
    in_maps = _shard_inputs(inputs)
    res = run_bass_kernel_spmd(nc, in_maps, core_ids=list(range(NCORES)))
    return np.concatenate([res.results[c]["out"] for c in range(NCORES)], axis=0)

